# revision 32
# baseline (speedup 1.0000x reference)
"""Trainium2 Bass kernel for nn_CDFVarianceLoss.

Math (per sample b, per tensor z in {pred[b], target[b]}, N = 65536):
    z' = (z - min z) / (max z - min z + 1e-6)
    h_j = sum_n exp(-(z'_n - c_j)^2 / (2*sigma^2)) + 1e-6,  c_j = j/63, j < 64
    cdf = cumsum(h / sum_j h)
    loss = mean_{b,j} (cdf_pred[b,j] - cdf_target[b,j])^2

Algorithmic core: the 64-bin soft histogram is a Gaussian KDE, so the
device only samples the KDE u_m = sum_n exp(-alpha (z'_n - y_m)^2) on a
coarse M=16 grid y (the KDE spectrum is sigma-limited, so a fixed
least-squares matrix A reconstructs all 64 bins: h ~= A u, ~1e-6 relative
loss error verified offline; bf16-quantized z adds ~7e-5).  This cuts the
O(N*BINS) ACT work 4x vs dense 64-bin evaluation.

Evaluation trick: Derivative_Erf(x) = 2/sqrt(pi) exp(-x^2), so the whole
Gaussian evaluation needs only a LINEAR PSUM input z (no z^2 / no bf16
hi-lo splits): ACT computes DErf(scale_p * z + bias_p) where the
per-partition scale_p = k s_u and bias_p = -k (s_u zmin_u + y_m), k =
sqrt(alpha), fold the minmax normalization in for free (the 2/sqrt(pi)
constant cancels in the cdf normalization).  The PE matmul is a one-hot
[8, 128] bf16 broadcast: partition 16 s + m holds raw z of slot s, so
each matmul column carries 8 elements (one per slot = (sample, tensor,
column-half)) and the core needs only 32768 ACT/PE columns total.

Distribution: data-parallel over the batch (2 samples/core), 8 cores;
device returns the raw KDE samples u [128] per core; the host applies A,
eps, normalize/cumsum/diff/square/mean in fp64 (A is 64x16; trivial
flops, same spirit as the baseline's host-side mean).
"""

import numpy as np

B = 16
N = 65536
BINS = 64
SIGMA = 0.05
EPS = 1e-6
ALPHA = 0.5 / SIGMA**2  # 200.0
KSC = float(np.sqrt(ALPHA))  # DErf arg scale
NCORES = 8
SPC = B // NCORES  # samples per core
P = 128
F = N // P  # 512 natural free dim
M = 16  # KDE grid points per (sample, tensor) unit
RPAD = 0.05  # grid span padding beyond [0, 1]
NSLOT = 8  # 4 units x 2 column halves
COLS = N // 2  # matmul columns per core (each col = 8 elements)
CHUNK = 8192  # columns per reshape tile
NCHUNK = COLS // CHUNK  # 4
MMN = 512  # matmul moving free dim
ACTB = 4  # matmuls per ACT block (PSUM tile = 4 banks)

_CACHE = {}


def _grid():
    return np.linspace(-RPAD, 1.0 + RPAD, M)


def _interp_matrix():
    """Least-squares fit: h(c_j) ~= sum_m A[j,m] u(y_m) for any empirical
    distribution of z in [0,1] (the KDE's spectrum is sigma-limited, so the
    coarse grid determines it; verified offline to ~1e-6 loss error)."""
    yg = _grid()
    c = np.linspace(0.0, 1.0, BINS)
    zt = np.linspace(0.0, 1.0, 8001)
    Bm = np.exp(-ALPHA * (zt[:, None] - yg[None, :]) ** 2)  # [T, M]
    G = np.exp(-ALPHA * (c[:, None] - zt[None, :]) ** 2)  # [BINS, T]
    A = np.linalg.solve(Bm.T @ Bm + 1e-9 * np.eye(M), Bm.T @ G.T).T
    return A  # [BINS, M] float64


def _build_nc():
    import concourse.bass as bass
    import concourse.bacc as bacc
    import concourse.tile as tile
    import ml_dtypes
    from concourse import mybir
    from concourse import bass_isa
    from contextlib import ExitStack

    f32 = mybir.dt.float32
    bf16 = mybir.dt.bfloat16
    AX = mybir.AxisListType
    OP = mybir.AluOpType
    ACTF = mybir.ActivationFunctionType

    nc = bacc.Bacc()
    pred_d = nc.declare_dram_parameter("pred", [SPC, N], f32, isOutput=False)
    targ_d = nc.declare_dram_parameter("target", [SPC, N], f32, isOutput=False)
    out_d = nc.declare_dram_parameter("u_out", [1, P], f32, isOutput=True)

    yg = _grid().astype(np.float32)
    # one-hot broadcast: slot s's z row -> partitions 16s..16s+16; replicated
    # at partition offsets 0/32/64/96 so rotating PE tile positions can each
    # load their stationary from the matching SBUF start partition
    lhsT_np = np.zeros((P, P), np.float32)
    for g in range(4):
        for s in range(NSLOT):
            lhsT_np[32 * g + s, M * s : M * s + M] = 1.0
    lhsT_np = lhsT_np.astype(ml_dtypes.bfloat16)
    # static -k*y_m term of the bias, tiled per slot
    ky_np = np.tile(KSC * yg, NSLOT).reshape(P, 1).astype(np.float32)

    lhsT_d = nc.inline_tensor(lhsT_np, name="lhsT_main")
    ky_d = nc.inline_tensor(ky_np, name="ky_col")
    ident_d = nc.inline_tensor(np.eye(P, dtype=np.float32), name="ident")

    with tile.TileContext(nc) as tc, ExitStack() as ctx:
        singles = ctx.enter_context(tc.tile_pool(name="singles", bufs=1))
        nat = ctx.enter_context(tc.tile_pool(name="nat", bufs=2))
        small = ctx.enter_context(tc.tile_pool(name="small", bufs=2))
        splits = ctx.enter_context(tc.tile_pool(name="splits", bufs=1))
        rows = ctx.enter_context(tc.tile_pool(name="rows", bufs=3))
        hp = ctx.enter_context(tc.tile_pool(name="hp", bufs=1))
        ps_pool = ctx.enter_context(tc.tile_pool(name="ps", bufs=2, space="PSUM"))

        # queue roles: sync + scalar carry the two input loads then the
        # reshape stream; gpsimd (SWDGE, a separate resource from the global
        # HWDGE that sync/scalar DMAs serialize on) carries consts + output.
        lhsT_sb = singles.tile([P, P], bf16)
        nc.gpsimd.dma_start(out=lhsT_sb, in_=lhsT_d[:, :])
        ky_sb = singles.tile([P, 1], f32)
        nc.gpsimd.dma_start(out=ky_sb, in_=ky_d[:, :])
        ident_sb = singles.tile([P, P], f32)
        nc.gpsimd.dma_start(out=ident_sb, in_=ident_d[:, :])
        # preload the DErf activation table off the critical path
        dummy = singles.tile([1, 1], f32)
        nc.scalar.activation(
            out=dummy, in_=ky_sb[0:1, 0:1], func=ACTF.Derivative_Erf,
            bias=ky_sb[0:1, 0:1], scale=1.0,
        )

        scale_col = hp.tile([P, 1], f32, name="scale_col")
        bias_col = hp.tile([P, 1], f32, name="bias_col")

        # one load per (tensor, sample) so the first samples' stats reduce
        # while the later loads are still in flight
        zu_tiles = []
        zb_tiles = []
        mm = small.tile([P, 8], f32, name="mm_all")
        for u in range(4):
            p, t = divmod(u, 2)
            src_d = pred_d if t == 0 else targ_d
            z = nat.tile([P, F], f32, tag=f"z{u}")
            ldq = nc.sync if t == 0 else nc.scalar
            ldq.dma_start(out=z, in_=src_d[p, :].rearrange("(p f) -> p f", p=P))
            zu_tiles.append(z)
        for u in range(4):
            z = zu_tiles[u]
            # per-partition (-min, max) into the batched [128, 8] stats tile
            nc.vector.tensor_reduce(
                out=mm[:, 2 * u : 2 * u + 1], in_=z, axis=AX.X, op=OP.min, negate=True
            )
            nc.vector.tensor_reduce(
                out=mm[:, 2 * u + 1 : 2 * u + 2], in_=z, axis=AX.X, op=OP.max
            )
            # raw z in bf16 feeds the PE (quantization verified offline);
            # cast on ACT (idle pre-stream) to keep DVE free for the reduces
            zb = splits.tile([P, F], bf16, tag=f"zb{u}")
            nc.scalar.copy(zb, z)
            zb_tiles.append(zb)
        mmr = small.tile([P, 8], f32, name="mmr_all")
        nc.gpsimd.partition_all_reduce(mmr, mm, P, bass_isa.ReduceOp.max)
        neg_min = mmr[:, :].rearrange("p (u c) -> p u c", c=2)[:, :, 0]  # [P, 4]
        gmax = mmr[:, :].rearrange("p (u c) -> p u c", c=2)[:, :, 1]
        r4 = small.tile([P, 4], f32, name="r4")
        nc.vector.tensor_tensor(out=r4, in0=gmax, in1=neg_min, op=OP.add)
        nc.vector.tensor_scalar_add(r4, r4, EPS)
        s4 = small.tile([P, 4], f32, name="s4")
        nc.vector.reciprocal(s4, r4)
        t14 = small.tile([P, 4], f32, name="t14")
        nc.vector.tensor_mul(t14, s4, neg_min)  # s * (-gmin)
        for u in range(4):
            blk = slice(32 * u, 32 * u + 32)
            nc.vector.tensor_scalar_mul(scale_col[blk, :], s4[blk, u : u + 1], KSC)
            nc.vector.tensor_scalar(
                bias_col[blk, :], t14[blk, u : u + 1], KSC, ky_sb[blk, 0:1],
                OP.mult, OP.subtract,
            )

        nblk = CHUNK // (ACTB * MMN)  # ACT blocks per chunk
        hparts = hp.tile([P, NCHUNK], f32, name="hparts")
        pp = CHUNK // F  # natural partitions per (chunk, half) slice (16)
        for ch in range(NCHUNK):
            rt = rows.tile([NSLOT, CHUNK], bf16, tag="rt")
            for u in range(4):
                for h in range(2):
                    p0 = 64 * h + pp * ch
                    # chunk0's DMAs split over two queues so the HWDGE
                    # serialization doesn't delay the stream start
                    q = nc.scalar if (ch == 0 and h == 1) else nc.sync
                    q.dma_start(
                        out=rt[2 * u + h : 2 * u + h + 1, :],
                        in_=zb_tiles[u][p0 : p0 + pp, :],
                    )
            for hb in range(nblk):
                ps = ps_pool.tile([P, ACTB * MMN], f32, tag="ps")
                for i in range(ACTB):
                    c0 = hb * ACTB * MMN + i * MMN
                    nc.tensor.matmul(
                        ps[:, i * MMN : (i + 1) * MMN],
                        lhsT_sb[0:NSLOT, :],
                        rt[:, c0 : c0 + MMN],
                        start=True,
                        stop=True,
                    )
                # accumulator read only on the last block of each chunk
                # (the ACT accumulator persists across activation insts)
                icol = ch * nblk + hb
                acc = hparts[:, ch : ch + 1] if hb == nblk - 1 else None
                nc.scalar.activation(
                    out=ps,
                    in_=ps,
                    func=ACTF.Derivative_Erf,
                    bias=bias_col[:, 0:1],
                    scale=scale_col[:, 0:1],
                    accum_out=acc,
                )

        uvec = small.tile([P, 1], f32, tag="uvec")
        nc.vector.tensor_reduce(out=uvec, in_=hparts, axis=AX.X, op=OP.add)
        # transpose u to a [1, 128] row so the output DMA is one contiguous
        # 512B descriptor (a [128, 1] column drains 128 tiny descriptors)
        urow_ps = ps_pool.tile([P, ACTB * MMN], f32, tag="ps")
        nc.tensor.transpose(urow_ps[0:1, 0:P], uvec, ident_sb[:, :])
        urow = small.tile([1, P], f32, tag="urowsb")
        nc.vector.tensor_copy(urow, urow_ps[0:1, 0:P])
        nc.sync.dma_start(out=out_d[0, :], in_=urow[0:1, :])

    nc.compile()
    return nc


def kernel(pred: np.ndarray, target: np.ndarray) -> np.ndarray:
    from concourse.bass_utils import run_bass_kernel_spmd

    if "nc" not in _CACHE:
        _CACHE["nc"] = _build_nc()
        _CACHE["A"] = _interp_matrix()
    nc = _CACHE["nc"]
    A = _CACHE["A"]

    pred = np.ascontiguousarray(np.asarray(pred, np.float32).reshape(B, N))
    target = np.ascontiguousarray(np.asarray(target, np.float32).reshape(B, N))
    in_maps = [
        {
            "pred": pred[i * SPC : (i + 1) * SPC],
            "target": target[i * SPC : (i + 1) * SPC],
        }
        for i in range(NCORES)
    ]
    res = run_bass_kernel_spmd(nc, in_maps, list(range(NCORES)))
    us = np.stack([r["u_out"][0] for r in res.results], axis=0)  # [8, 128]
    us = us.astype(np.float64).reshape(NCORES, 4, 2, M)  # [core, unit, half, M]
    u = us.sum(axis=2).reshape(NCORES * 2, 2, M)  # [16 samples, pred/targ, M]
    # the DErf 2/sqrt(pi) constant cancels in the cdf normalization
    h = u @ A.T + EPS
    cdf = np.cumsum(h / h.sum(axis=-1, keepdims=True), axis=-1)
    return np.float32(np.mean((cdf[:, 0] - cdf[:, 1]) ** 2))


# revision 34
# speedup vs baseline: 1.0446x; 1.0446x over previous
"""Trainium2 Bass kernel for nn_CDFVarianceLoss.

Math (per sample b, per tensor z in {pred[b], target[b]}, N = 65536):
    z' = (z - min z) / (max z - min z + 1e-6)
    h_j = sum_n exp(-(z'_n - c_j)^2 / (2*sigma^2)) + 1e-6,  c_j = j/63, j < 64
    cdf = cumsum(h / sum_j h)
    loss = mean_{b,j} (cdf_pred[b,j] - cdf_target[b,j])^2

Algorithmic core: the 64-bin soft histogram is a Gaussian KDE, so the
device only samples the KDE u_m = sum_n exp(-alpha (z'_n - y_m)^2) on a
coarse M=16 grid y (the KDE spectrum is sigma-limited, so a fixed
least-squares matrix A reconstructs all 64 bins: h ~= A u, ~1e-6 relative
loss error verified offline; bf16-quantized z adds ~7e-5).  This cuts the
O(N*BINS) ACT work 4x vs dense 64-bin evaluation.

Evaluation trick: Derivative_Erf(x) = 2/sqrt(pi) exp(-x^2), so the whole
Gaussian evaluation needs only a LINEAR PSUM input z (no z^2 / no bf16
hi-lo splits): ACT computes DErf(scale_p * z + bias_p) where the
per-partition scale_p = k s_u and bias_p = -k (s_u zmin_u + y_m), k =
sqrt(alpha), fold the minmax normalization in for free (the 2/sqrt(pi)
constant cancels in the cdf normalization).  The PE matmul is a one-hot
[8, 128] bf16 broadcast: partition 16 s + m holds raw z of slot s, so
each matmul column carries 8 elements (one per slot = (sample, tensor,
column-half)) and the core needs only 32768 ACT/PE columns total.

Distribution: data-parallel over the batch (2 samples/core), 8 cores;
device returns the raw KDE samples u [128] per core; the host applies A,
eps, normalize/cumsum/diff/square/mean in fp64 (A is 64x16; trivial
flops, same spirit as the baseline's host-side mean).
"""

import numpy as np

B = 16
N = 65536
BINS = 64
SIGMA = 0.05
EPS = 1e-6
ALPHA = 0.5 / SIGMA**2  # 200.0
KSC = float(np.sqrt(ALPHA))  # DErf arg scale
NCORES = 8
SPC = B // NCORES  # samples per core
P = 128
F = N // P  # 512 natural free dim
M = 16  # KDE grid points per (sample, tensor) unit
RPAD = 0.05  # grid span padding beyond [0, 1]
NSLOT = 8  # 4 units x 2 column halves
COLS = N // 2  # matmul columns per core (each col = 8 elements)
CHUNK = 8192  # columns per reshape tile
NCHUNK = COLS // CHUNK  # 4
MMN = 512  # matmul moving free dim
ACTB = 4  # matmuls per ACT block (PSUM tile = 4 banks)

_CACHE = {}


def _grid():
    return np.linspace(-RPAD, 1.0 + RPAD, M)


def _interp_matrix():
    """Least-squares fit: h(c_j) ~= sum_m A[j,m] u(y_m) for any empirical
    distribution of z in [0,1] (the KDE's spectrum is sigma-limited, so the
    coarse grid determines it; verified offline to ~1e-6 loss error)."""
    yg = _grid()
    c = np.linspace(0.0, 1.0, BINS)
    zt = np.linspace(0.0, 1.0, 8001)
    Bm = np.exp(-ALPHA * (zt[:, None] - yg[None, :]) ** 2)  # [T, M]
    G = np.exp(-ALPHA * (c[:, None] - zt[None, :]) ** 2)  # [BINS, T]
    A = np.linalg.solve(Bm.T @ Bm + 1e-9 * np.eye(M), Bm.T @ G.T).T
    return A  # [BINS, M] float64


def _build_nc():
    import concourse.bass as bass
    import concourse.bacc as bacc
    import concourse.tile as tile
    import ml_dtypes
    from concourse import mybir
    from concourse import bass_isa
    from contextlib import ExitStack

    f32 = mybir.dt.float32
    bf16 = mybir.dt.bfloat16
    AX = mybir.AxisListType
    OP = mybir.AluOpType
    ACTF = mybir.ActivationFunctionType

    nc = bacc.Bacc()
    pred_d = nc.declare_dram_parameter("pred", [SPC, N], f32, isOutput=False)
    targ_d = nc.declare_dram_parameter("target", [SPC, N], f32, isOutput=False)
    out_d = nc.declare_dram_parameter("u_out", [1, P], f32, isOutput=True)

    yg = _grid().astype(np.float32)
    # one-hot broadcast: slot s's z row -> partitions 16s..16s+16; replicated
    # at partition offsets 0/32/64/96 so rotating PE tile positions can each
    # load their stationary from the matching SBUF start partition
    lhsT_np = np.zeros((P, P), np.float32)
    for g in range(4):
        for s in range(NSLOT):
            lhsT_np[32 * g + s, M * s : M * s + M] = 1.0
    lhsT_np = lhsT_np.astype(ml_dtypes.bfloat16)
    # static -k*y_m term of the bias, tiled per slot
    ky_np = np.tile(KSC * yg, NSLOT).reshape(P, 1).astype(np.float32)

    lhsT_d = nc.inline_tensor(lhsT_np, name="lhsT_main")
    ky_d = nc.inline_tensor(ky_np, name="ky_col")
    ident_d = nc.inline_tensor(np.eye(P, dtype=np.float32), name="ident")

    with tile.TileContext(nc) as tc, ExitStack() as ctx:
        singles = ctx.enter_context(tc.tile_pool(name="singles", bufs=1))
        nat = ctx.enter_context(tc.tile_pool(name="nat", bufs=2))
        small = ctx.enter_context(tc.tile_pool(name="small", bufs=2))
        splits = ctx.enter_context(tc.tile_pool(name="splits", bufs=1))
        rows = ctx.enter_context(tc.tile_pool(name="rows", bufs=3))
        hp = ctx.enter_context(tc.tile_pool(name="hp", bufs=1))
        ps_pool = ctx.enter_context(tc.tile_pool(name="ps", bufs=2, space="PSUM"))

        # queue roles: sync + scalar carry the two input loads then the
        # reshape stream; gpsimd (SWDGE, a separate resource from the global
        # HWDGE that sync/scalar DMAs serialize on) carries consts + output.
        lhsT_sb = singles.tile([P, P], bf16)
        nc.gpsimd.dma_start(out=lhsT_sb, in_=lhsT_d[:, :])
        ky_sb = singles.tile([P, 1], f32)
        nc.gpsimd.dma_start(out=ky_sb, in_=ky_d[:, :])
        ident_sb = singles.tile([P, P], f32)
        nc.gpsimd.dma_start(out=ident_sb, in_=ident_d[:, :])
        # preload the DErf activation table off the critical path; scale=100
        # makes the output exactly 0 so the persistent accumulator (read by
        # the stream's per-chunk accum_out) is not polluted
        dummy = singles.tile([1, 1], f32)
        nc.scalar.activation(
            out=dummy, in_=ky_sb[0:1, 0:1], func=ACTF.Derivative_Erf,
            bias=ky_sb[0:1, 0:1], scale=100.0,
        )

        scale_col = hp.tile([P, 1], f32, name="scale_col")
        bias_col = hp.tile([P, 1], f32, name="bias_col")

        # one load per (tensor, sample) so the first samples' stats reduce
        # while the later loads are still in flight
        zu_tiles = []
        zb_tiles = []
        mm = small.tile([P, 8], f32, name="mm_all")
        for u in range(4):
            p, t = divmod(u, 2)
            src_d = pred_d if t == 0 else targ_d
            z = nat.tile([P, F], f32, tag=f"z{u}")
            ldq = nc.sync if t == 0 else nc.scalar
            ldq.dma_start(out=z, in_=src_d[p, :].rearrange("(p f) -> p f", p=P))
            zu_tiles.append(z)
        for u in range(4):
            z = zu_tiles[u]
            # per-partition (-min, max) into the batched [128, 8] stats tile
            nc.vector.tensor_reduce(
                out=mm[:, 2 * u : 2 * u + 1], in_=z, axis=AX.X, op=OP.min, negate=True
            )
            nc.vector.tensor_reduce(
                out=mm[:, 2 * u + 1 : 2 * u + 2], in_=z, axis=AX.X, op=OP.max
            )
            # raw z in bf16 feeds the PE (quantization verified offline);
            # NOT on ACT: every ACT op's row sums drain into the persistent
            # accumulator the stream reads per chunk
            zb = splits.tile([P, F], bf16, tag=f"zb{u}")
            nc.vector.tensor_copy(zb, z)
            zb_tiles.append(zb)
        mmr = small.tile([P, 8], f32, name="mmr_all")
        nc.gpsimd.partition_all_reduce(mmr, mm, P, bass_isa.ReduceOp.max)
        neg_min = mmr[:, :].rearrange("p (u c) -> p u c", c=2)[:, :, 0]  # [P, 4]
        gmax = mmr[:, :].rearrange("p (u c) -> p u c", c=2)[:, :, 1]
        r4 = small.tile([P, 4], f32, name="r4")
        nc.vector.tensor_tensor(out=r4, in0=gmax, in1=neg_min, op=OP.add)
        nc.vector.tensor_scalar_add(r4, r4, EPS)
        s4 = small.tile([P, 4], f32, name="s4")
        nc.vector.reciprocal(s4, r4)
        t14 = small.tile([P, 4], f32, name="t14")
        nc.vector.tensor_mul(t14, s4, neg_min)  # s * (-gmin)
        for u in range(4):
            blk = slice(32 * u, 32 * u + 32)
            nc.vector.tensor_scalar_mul(scale_col[blk, :], s4[blk, u : u + 1], KSC)
            nc.vector.tensor_scalar(
                bias_col[blk, :], t14[blk, u : u + 1], KSC, ky_sb[blk, 0:1],
                OP.mult, OP.subtract,
            )

        nblk = CHUNK // (ACTB * MMN)  # ACT blocks per chunk
        hparts = hp.tile([P, NCHUNK], f32, name="hparts")
        pp = CHUNK // F  # natural partitions per (chunk, half) slice (16)
        for ch in range(NCHUNK):
            rt = rows.tile([NSLOT, CHUNK], bf16, tag="rt")
            for u in range(4):
                for h in range(2):
                    p0 = 64 * h + pp * ch
                    # chunk0's DMAs split over two queues so the HWDGE
                    # serialization doesn't delay the stream start
                    q = nc.scalar if (ch == 0 and h == 1) else nc.sync
                    q.dma_start(
                        out=rt[2 * u + h : 2 * u + h + 1, :],
                        in_=zb_tiles[u][p0 : p0 + pp, :],
                    )
            for hb in range(nblk):
                ps = ps_pool.tile([P, ACTB * MMN], f32, tag="ps")
                for i in range(ACTB):
                    c0 = hb * ACTB * MMN + i * MMN
                    nc.tensor.matmul(
                        ps[:, i * MMN : (i + 1) * MMN],
                        lhsT_sb[0:NSLOT, :],
                        rt[:, c0 : c0 + MMN],
                        start=True,
                        stop=True,
                    )
                # accumulator read only on the last block of each chunk
                # (the ACT accumulator persists across activation insts)
                icol = ch * nblk + hb
                acc = hparts[:, ch : ch + 1] if hb == nblk - 1 else None
                nc.scalar.activation(
                    out=ps,
                    in_=ps,
                    func=ACTF.Derivative_Erf,
                    bias=bias_col[:, 0:1],
                    scale=scale_col[:, 0:1],
                    accum_out=acc,
                )

        uvec = small.tile([P, 1], f32, tag="uvec")
        nc.vector.tensor_reduce(out=uvec, in_=hparts, axis=AX.X, op=OP.add)
        # transpose u to a [1, 128] row so the output DMA is one contiguous
        # 512B descriptor (a [128, 1] column drains 128 tiny descriptors)
        urow_ps = ps_pool.tile([P, ACTB * MMN], f32, tag="ps")
        nc.tensor.transpose(urow_ps[0:1, 0:P], uvec, ident_sb[:, :])
        urow = small.tile([1, P], f32, tag="urowsb")
        nc.vector.tensor_copy(urow, urow_ps[0:1, 0:P])
        nc.sync.dma_start(out=out_d[0, :], in_=urow[0:1, :])

    nc.compile()
    return nc


def kernel(pred: np.ndarray, target: np.ndarray) -> np.ndarray:
    from concourse.bass_utils import run_bass_kernel_spmd

    if "nc" not in _CACHE:
        _CACHE["nc"] = _build_nc()
        _CACHE["A"] = _interp_matrix()
    nc = _CACHE["nc"]
    A = _CACHE["A"]

    pred = np.ascontiguousarray(np.asarray(pred, np.float32).reshape(B, N))
    target = np.ascontiguousarray(np.asarray(target, np.float32).reshape(B, N))
    in_maps = [
        {
            "pred": pred[i * SPC : (i + 1) * SPC],
            "target": target[i * SPC : (i + 1) * SPC],
        }
        for i in range(NCORES)
    ]
    res = run_bass_kernel_spmd(nc, in_maps, list(range(NCORES)))
    us = np.stack([r["u_out"][0] for r in res.results], axis=0)  # [8, 128]
    us = us.astype(np.float64).reshape(NCORES, 4, 2, M)  # [core, unit, half, M]
    u = us.sum(axis=2).reshape(NCORES * 2, 2, M)  # [16 samples, pred/targ, M]
    # the DErf 2/sqrt(pi) constant cancels in the cdf normalization
    h = u @ A.T + EPS
    cdf = np.cumsum(h / h.sum(axis=-1, keepdims=True), axis=-1)
    return np.float32(np.mean((cdf[:, 0] - cdf[:, 1]) ** 2))
